# revision 1
# baseline (speedup 1.0000x reference)
"""Trainium2 Bass kernel for the Brill-Lindquist Christoffel-symbol grid.

Math: the reference reduces to
    psi  = 1 + sum_n m_n / (2 r_n),   m = softplus(pre)
    h    = psi^4                       (= exp(4*log(psi)))
    G_c  = finite-difference gradient of h along grid axis c (2nd order
           central interior, 1st order one-sided edges, spacing DX)
    W_c  = 0.5 * G_c / h
    Gamma^i_{jk} = delta_ij W_k + delta_ik W_j - delta_jk W_i
so the [96,96,96,3,3,3] output is +-W_c scattered over 27 slots per point.

Sharding: axis 0 (12 planes per core x 8 cores). h is analytic in the
inputs, so each core evaluates its slab plus a 1-plane halo directly --
no inter-core exchange. Per core the grid is row-packed: row = a0*96+a1
(1152 rows -> 9 tiles of 128 partitions), free dim = a2 (96); h lives on
an 11-tile extended row window (halo tiles at both ends).

Axis-0/1 derivatives: SBUF->SBUF DMAs build row-shifted copies of the h
field (+-96 rows for axis 0, +-1 row for axis 1); the derivative is then
an aligned elementwise subtract. Grid-edge one-sided differences are
restored by rewriting the difference rows as 2*(h_shift - h) (ghost-cell
linear extrapolation), grouped into a few wide APs; the a0 edges exist
only on cores 0/7 and are applied through a per-core 0/1 mask with
copy_predicated. The axis-2 derivative uses shifted free-dim slices.

Output assembly: W is written (a2,c)-interleaved; the 9 diagonal slots
fill with one 4-dim-AP copy, the 12 off-diagonal nonzeros with 6
paired-slot ops (stride-0 source broadcast), zeros persist in reused
output buffers. Output DMA is contiguous per row (10368 B).
"""

import numpy as np

RES = 96
N_CORES = 8
PLANES = RES // N_CORES        # 12
LROWS = PLANES * RES           # 1152 local rows
NT = LROWS // 128              # 9 local 128-row tiles
EXTNT = NT + 2                 # 11 extended tiles (halo)
NROWS_G = RES * RES            # 9216 global rows
S27 = 27
NOB = 3                        # rotating output buffers

# Broadcast-row layout (free offsets in the [128, BCW] broadcast tile)
B_POS = 0        # px1,py1,pz1,px2,py2,pz2
B_S = 6          # m1/2, m2/2
B_CROW = 8       # (z - pz1)^2 [96], (z - pz2)^2 [96]
B_KVEC = 200     # z-FD column scale [96]
BCW = 296

# a1-edge fixup groups: (partition, first block, nblocks step 3)
A1LO_GROUPS = [(0, 0), (96, 0), (64, 1), (32, 2)]    # rows with a1 == 0
A1HI_GROUPS = [(95, 0), (63, 1), (31, 2), (127, 2)]  # rows with a1 == 95


def _grid_x():
    # Match the reference grid bit-for-bit: jnp.linspace in fp32 on CPU
    # (the reference's softplus cannot compile for the neuron backend, so
    # it necessarily runs on the jax CPU platform).
    import jax
    import jax.numpy as jnp
    MAX_X = 1.0
    DX = np.float32(MAX_X / (RES / 2 - 1))

    def _ls():
        return jnp.linspace(
            DX * (1 - RES / 2), DX * (RES / 2 - 1), RES, dtype=jnp.float32
        )

    try:
        with jax.default_device(jax.devices("cpu")[0]):
            x = np.asarray(_ls())
    except Exception:
        x = np.asarray(_ls())
    return x, float(DX)


def _fd_sources(idx, coeff_c, coeff_e):
    """(offset, coeff) pairs for d/didx with 1st-order one-sided edges."""
    if idx == 0:
        return [(1, coeff_e), (0, -coeff_e)]
    if idx == RES - 1:
        return [(0, coeff_e), (-1, -coeff_e)]
    return [(1, coeff_c), (-1, -coeff_c)]


def _build_dmat(core, DX):
    """[128, 6*3*128] bf16 FD matrices as matmul lhsT ([q, p] = coeff of
    ext-row q in output row p); 0.5 Christoffel factor folded in. All
    values are +-0.25/DX or +-0.5/DX = +-11.75 / +-23.5, exact in bf16.
    Entries: 0 g0(t=0), 1 g0(interior), 2 g0(t=8), 3..5 g1(t%3)."""
    import ml_dtypes
    c0 = 0.5 * (1.0 / (2.0 * np.float64(DX)))
    ce = 0.5 * (1.0 / np.float64(DX))
    out = np.zeros((128, 6 * 3 * 128), np.float64)

    def fill(entry, t, axis):
        for p in range(128):
            gr = core * LROWS + 128 * t + p
            a = (gr // RES) if axis == 0 else (gr % RES)
            step = RES if axis == 0 else 1
            for off, cf in _fd_sources(a, c0, ce):
                g2 = gr + off * step
                e_ = g2 - core * LROWS + 128
                j = e_ // 128 - t
                q = e_ - 128 * (t + j)
                assert 0 <= j <= 2 and 0 <= q < 128, (core, t, p, off)
                out[q, (entry * 3 + j) * 128 + p] = cf

    fill(0, 0, 0)
    fill(1, 1, 0)
    fill(2, NT - 1, 0)
    for v in range(3):
        fill(3 + v, v, 1)
    return out.astype(ml_dtypes.bfloat16)


def _build_static(core, x, DX):
    slab = core * LROWS
    e = np.arange(EXTNT * 128)
    g = np.clip(slab - 128 + e, 0, NROWS_G - 1)   # clamp halo overrun (unused rows)
    xcol = x[g % RES].reshape(EXTNT, 128).T.copy()     # X coordinate (a1)
    ycol = x[g // RES].reshape(EXTNT, 128).T.copy()    # Y coordinate (a0)
    kvec = np.full(RES, 0.25 / DX, np.float64)
    kvec[0] = kvec[-1] = 0.5 / DX
    return {
        "xcol": np.ascontiguousarray(xcol, np.float32),
        "ycol": np.ascontiguousarray(ycol, np.float32),
        "zrow": x.reshape(1, RES).astype(np.float32),
        "kvec": kvec.reshape(1, RES).astype(np.float32),
        "dmat": _build_dmat(core, DX),
    }


def _build_program(DX):
    import dataclasses as _dc

    import concourse.bacc as bacc
    import concourse.mybir as mybir
    import concourse.tile as tile

    DT = mybir.dt.float32
    BF = mybir.dt.bfloat16
    AF = mybir.ActivationFunctionType

    nc = bacc.Bacc(None, target_bir_lowering=False, debug=True)
    d_pos = nc.dram_tensor("bh_pos", [1, 6], DT, kind="ExternalInput")
    d_m = nc.dram_tensor("bh_m", [1, 2], DT, kind="ExternalInput")
    d_xcol = nc.dram_tensor("xcol", [128, EXTNT], DT, kind="ExternalInput")
    d_ycol = nc.dram_tensor("ycol", [128, EXTNT], DT, kind="ExternalInput")
    d_zrow = nc.dram_tensor("zrow", [1, RES], DT, kind="ExternalInput")
    d_kvec = nc.dram_tensor("kvec", [1, RES], DT, kind="ExternalInput")
    d_dmat = nc.dram_tensor("dmat", [128, 6 * 3 * 128], BF, kind="ExternalInput")
    d_out = nc.dram_tensor("out", [LROWS, RES * S27], DT, kind="ExternalOutput")

    HW_ = EXTNT * RES             # 1056: free width of the ext h field
    HCHUNKS = [(0, 3), (3, 6), (6, 9), (9, 11)]   # ext-block ranges
    with tile.TileContext(nc) as tc:
        with (
            tc.tile_pool(name="const", bufs=1) as cpool,
            tc.tile_pool(name="work", bufs=3) as wpool,
            tc.tile_pool(name="wout", bufs=3) as wopool,
            tc.tile_pool(name="obuf", bufs=NOB) as opool,
            tc.tile_pool(name="psum", bufs=2, space="PSUM") as pspool,
            tc.tile_pool(name="psb", bufs=1, space="PSUM") as psbpool,
        ):
            # --- constants in ---
            dm = cpool.tile([128, 6 * 3 * 128], BF)
            nc.sync.dma_start(dm[:], d_dmat[:])
            xc = cpool.tile([128, EXTNT], DT)
            nc.sync.dma_start(xc[:], d_xcol[:])
            yc = cpool.tile([128, EXTNT], DT)
            nc.sync.dma_start(yc[:], d_ycol[:])
            zr = cpool.tile([1, RES], DT)
            nc.sync.dma_start(zr[:], d_zrow[:])

            # --- broadcast row R: pos | m/2 | (z-pz)^2 x2 | kvec ---
            R = cpool.tile([1, BCW], DT)
            nc.sync.dma_start(R[:, B_POS:B_POS + 6], d_pos[:])
            nc.sync.dma_start(R[:, B_KVEC:B_KVEC + RES], d_kvec[:])
            m = cpool.tile([1, 2], DT)
            nc.sync.dma_start(m[:], d_m[:])
            nc.vector.tensor_scalar_mul(R[:, B_S:B_S + 2], m[:], 0.5)
            for n in range(2):
                dzn = cpool.tile([1, RES], DT, tag="dzn")
                nc.vector.tensor_scalar_sub(dzn[:], zr[:], R[:, 2 + 3 * n:3 + 3 * n])
                nc.vector.tensor_mul(
                    R[:, B_CROW + RES * n:B_CROW + RES * (n + 1)], dzn[:], dzn[:]
                )
            ones = cpool.tile([1, 128], DT)
            nc.vector.memset(ones[:], 1.0)
            bps = psbpool.tile([128, BCW], DT)
            nc.tensor.matmul(bps[:], ones[:], R[:])
            Bb = cpool.tile([128, BCW], DT)
            nc.vector.tensor_copy(Bb[:], bps[:])

            # --- per-partition (x-px)^2+(y-py)^2 for the 11 ext tiles ---
            ab = []
            for n in range(2):
                dxn = cpool.tile([128, EXTNT], DT, tag="dxn")
                nc.vector.tensor_scalar_sub(dxn[:], xc[:], Bb[:, 3 * n:3 * n + 1])
                dyn = cpool.tile([128, EXTNT], DT, tag="dyn")
                nc.vector.tensor_scalar_sub(dyn[:], yc[:], Bb[:, 3 * n + 1:3 * n + 2])
                nc.vector.tensor_mul(dxn[:], dxn[:], dxn[:])
                nc.vector.tensor_mul(dyn[:], dyn[:], dyn[:])
                abn = cpool.tile([128, EXTNT], DT, tag=f"ab{n}")
                nc.vector.tensor_add(abn[:], dxn[:], dyn[:])
                ab.append(abn)

            # --- h = psi^4 on the extended field + 3-way bf16 split ---
            # psi = 1 + (mh1*r2 + mh2*r1)/(r1*r2); h = ((psi)^2)^2
            # processed in 3-block-wide chunks to amortize per-op overhead
            H = cpool.tile([128, HW_], DT)
            Hh = cpool.tile([128, HW_], BF)
            Hm = cpool.tile([128, HW_], BF)
            Hl = cpool.tile([128, HW_], BF)
            for b0, b1 in HCHUNKS:
                nb = b1 - b0
                W = nb * RES
                csl = slice(RES * b0, RES * b1)
                rr = []
                for n in range(2):
                    r2 = wpool.tile([128, W], DT, tag="r2")
                    r2v = r2[:].rearrange("p (b z) -> p b z", z=RES)
                    crow = Bb[:, B_CROW + RES * n:B_CROW + RES * (n + 1)]
                    crow_b = _dc.replace(crow, ap=[crow.ap[0], [0, nb], [1, RES]])
                    absl = ab[n][:, b0:b1]
                    ab_b = _dc.replace(absl, ap=[absl.ap[0], [1, nb], [0, RES]])
                    nc.gpsimd.tensor_add(r2v[:, :, :], crow_b, ab_b)
                    rn = wpool.tile([128, W], DT, tag=f"rr{n}")
                    nc.scalar.activation(rn[:], r2[:], AF.Sqrt)
                    rr.append(rn)
                v = wpool.tile([128, W], DT, tag="v")
                nc.gpsimd.tensor_mul(v[:], rr[0][:], rr[1][:])
                u1 = wpool.tile([128, W], DT, tag="u1")
                nc.scalar.mul(u1[:], rr[1][:], Bb[:, B_S:B_S + 1])
                u2 = wpool.tile([128, W], DT, tag="u2")
                nc.scalar.mul(u2[:], rr[0][:], Bb[:, B_S + 1:B_S + 2])
                u = wpool.tile([128, W], DT, tag="u")
                nc.gpsimd.tensor_add(u[:], u1[:], u2[:])
                vinv = wpool.tile([128, W], DT, tag="vinv")
                vscr = wpool.tile([128, W], DT, tag="vscr")
                nc.vector.reciprocal_approx_accurate(vinv[:], v[:], vscr[:])
                psim = wpool.tile([128, W], DT, tag="psim")
                nc.vector.tensor_mul(psim[:], u[:], vinv[:])
                hsq = wpool.tile([128, W], DT, tag="hsq")
                nc.scalar.activation(hsq[:], psim[:], AF.Square, bias=1.0)
                nc.scalar.activation(H[:, csl], hsq[:], AF.Square)
                # 3-way bf16 split: h = hi + mid + lo (+ O(2^-27 h))
                nc.scalar.copy(Hh[:, csl], H[:, csl])
                s1 = wpool.tile([128, W], DT, tag="s1")
                nc.gpsimd.tensor_sub(s1[:], H[:, csl], Hh[:, csl])
                nc.scalar.copy(Hm[:, csl], s1[:])
                s2 = wpool.tile([128, W], DT, tag="s2")
                nc.gpsimd.tensor_sub(s2[:], s1[:], Hm[:, csl])
                nc.vector.tensor_copy(Hl[:, csl], s2[:])

            # --- rotating output buffers, zero slots pre-filled once ---
            otiles = []
            for i in range(NOB):
                O = opool.tile([128, RES * S27], DT, tag=f"ob{i}")
                O3 = O[:].rearrange("p (z s) -> p z s", s=S27)
                nc.gpsimd.memset(O3[:, :, 5:8:2], 0.0)
                nc.gpsimd.memset(O3[:, :, 11:20:4], 0.0)
                nc.gpsimd.memset(O3[:, :, 21], 0.0)
                otiles.append(O)

            # --- per local tile: FD matmuls, W, scatter, store ---
            for t in range(NT):
                g0e = 0 if t == 0 else (2 if t == NT - 1 else 1)
                g1e = 3 + (t % 3)
                hsl = slice(RES * (t + 1), RES * (t + 2))
                p0 = pspool.tile([128, RES], DT, tag="p0")
                p1 = pspool.tile([128, RES], DT, tag="p1")
                for ge, pp in ((g0e, p0), (g1e, p1)):
                    k = 0
                    for j in range(3):
                        lhs = dm[:, (ge * 3 + j) * 128:(ge * 3 + j + 1) * 128]
                        rsl = slice(RES * (t + j), RES * (t + j + 1))
                        for Hs in (Hh, Hm, Hl):
                            nc.tensor.matmul(
                                pp[:], lhs, Hs[:, rsl], start=(k == 0), stop=(k == 8)
                            )
                            k += 1

                hinv = wopool.tile([128, RES], DT, tag="hinv")
                nc.vector.reciprocal_approx_fast(hinv[:], H[:, hsl])
                hz = wopool.tile([128, RES], DT, tag="hz")
                nc.gpsimd.tensor_mul(hz[:], hinv[:], Bb[:, B_KVEC:B_KVEC + RES])

                w3 = wopool.tile([128, 3 * RES], DT, tag="w3")
                W3v = w3[:].rearrange("p (z c) -> p z c", c=3)
                nc.vector.tensor_mul(W3v[:, :, 0], p0[:], hinv[:])
                nc.vector.tensor_mul(W3v[:, :, 1], p1[:], hinv[:])
                st = wopool.tile([128, RES], DT, tag="st")
                nc.gpsimd.tensor_sub(st[:, 1:95], H[:, hsl][:, 2:96], H[:, hsl][:, 0:94])
                nc.gpsimd.tensor_sub(st[:, 0:1], H[:, hsl][:, 1:2], H[:, hsl][:, 0:1])
                nc.gpsimd.tensor_sub(
                    st[:, 95:96], H[:, hsl][:, 95:96], H[:, hsl][:, 94:95]
                )
                nc.vector.tensor_mul(W3v[:, :, 2], st[:], hz[:])

                O = otiles[t % NOB]
                O3 = O[:].rearrange("p (z s) -> p z s", s=S27)
                # 9 diagonal slots (i==j rows) in one op: slot a2*27+12i+c
                ddst = _dc.replace(
                    O[:], ap=[O[:].ap[0], [S27, RES], [12, 3], [1, 3]]
                )
                dsrc = _dc.replace(
                    w3[:], ap=[w3[:].ap[0], [3, RES], [0, 3], [1, 3]]
                )
                nc.scalar.copy(ddst, dsrc)
                # remaining 12 nonzero slots: 6 paired-slot ops
                for (a, b, c, sg) in (
                    (10, 20, 0, 1), (3, 23, 1, 1), (6, 16, 2, 1),
                    (4, 8, 0, -1), (9, 17, 1, -1), (18, 22, 2, -1),
                ):
                    dst = O3[:, :, a:b + 1:b - a]
                    src = _dc.replace(
                        W3v[:, :, c], ap=W3v[:, :, c].ap + [[0, 2]]
                    )
                    if sg > 0:
                        nc.vector.tensor_copy(dst, src)
                    else:
                        nc.vector.tensor_scalar_mul(dst, src, -1.0)
                nc.sync.dma_start(d_out[128 * t:128 * (t + 1), :], O[:])

    nc.finalize()
    return nc


_CACHE = {}


def _get_setup():
    if "nc" not in _CACHE:
        x, DX = _grid_x()
        _CACHE["static"] = [_build_static(c, x, DX) for c in range(N_CORES)]
        _CACHE["nc"] = _build_program(DX)
    return _CACHE["nc"], _CACHE["static"]


def kernel(BH_positions, BH_masses_presoftplus):
    from concourse.bass_utils import run_bass_kernel_spmd

    nc, static = _get_setup()
    pos = np.ascontiguousarray(np.asarray(BH_positions, np.float32).reshape(1, 6))
    # softplus of the two mass parameters (log1p(exp(x)) in fp32, as jax.nn.softplus)
    pre = np.asarray(BH_masses_presoftplus, np.float32)
    masses = np.log1p(np.exp(pre)).astype(np.float32).reshape(1, 2)
    in_maps = [{"bh_pos": pos, "bh_m": masses, **static[c]} for c in range(N_CORES)]
    res = run_bass_kernel_spmd(nc, in_maps, list(range(N_CORES)))
    parts = [
        res.results[c]["out"].reshape(PLANES, RES, RES, 3, 3, 3)
        for c in range(N_CORES)
    ]
    return np.ascontiguousarray(np.concatenate(parts, axis=0))

